# revision 45
# baseline (speedup 1.0000x reference)
"""Trainium2 Bass kernel for a dense transformer block (B=4, T=2048, D=1024, H=16).

Sharding: 8 cores = 4 batches x 2-way head split for attention, then 2-way
token split (within each batch pair) for the attention projection + MLP.
Cross-core communication: four per-group pairwise AllGathers of normalized
per-head attention outputs (bf16), issued inline so they overlap attention.

v2: fp8-e4m3 DoubleRow matmuls (2x PE throughput) for every contraction over
d_model / d_ff (QKV, attn proj, MLP up/down) and for attn@V (key-tile pairs
as DoubleRow planes). Scores stay bf16 (small-K DR is not faster). Weights
are pre-scaled x256 into fp8 on the host; activations are scaled x16 (x32
for V / exp) with the descale folded into activation-engine evacuations.

Per-core dataflow (b = core//2, heads = 8*(core%2) .. +8):
  LN1 per 512-token block -> x1bf bf16 (residual) + x1 fp8 -> PE-transpose
    -> x1T fp8 [D, T]; QT/KT (bf16, x4096) + V (fp8 x32) interleaved per block
  x1own bf16 residual half extracted with one rank-indexed SBUF DMA
  attention per group g: S^T = KT.T @ QT (bf16, 2 heads quadrant-packed),
    exp on ACT (all scales folded, output fp8 x64), causal 0/1 mask muls,
    attnT[65, q] accumulated with fp8 DR over key-tile pairs (row 64 = Z)
  -> stage_g -> AllGather_g (issued right after group g stages)
  proj + residual + LN2 -> x3bf bf16 + x3T fp8; MLP up (DR, relu on ACT,
  hT fp8 x32); MLP down (DR, w2 streamed once), +x3 residual, out f32.
"""

import os
import sys

for _p in ("/opt/trn_rl_repo", "/root/.axon_site/_ro/trn_rl_repo"):
    if os.path.isdir(_p) and _p not in sys.path:
        sys.path.insert(0, _p)

from contextlib import ExitStack

import ml_dtypes
import numpy as np

import concourse.bass as bass
import concourse.mybir as mybir
import concourse.tile as tile
from concourse import bacc
from concourse.masks import make_identity

F32 = mybir.dt.float32
BF16 = mybir.dt.bfloat16
F8 = mybir.dt.float8e4
AF = mybir.ActivationFunctionType
ALU = mybir.AluOpType
DR = mybir.MatmulPerfMode.DoubleRow

N_CORES = 8
EPS = 1e-5
WS = 256.0      # weight fp8 scale
XS = 16.0       # activation fp8 scale
ES = 1.0        # exp output scale (bf16)
VS = 32.0       # V fp8 scale


def build_program(T=2048, D=1024, DFF=4096, mask_mode="causal", flags=()):
    """Emit the SPMD program (identical across cores; all per-core variation
    comes in via input data + cc_rank-indexed DMA). Returns compiled Bacc."""
    flags = set(flags)
    DC = D // 128            # d-model 128-chunks
    DP = DC // 2             # d-model 256-pairs (DoubleRow)
    HS = DFF // 128          # mlp hidden 128-slices
    HP = HS // 2             # mlp hidden 256-pairs
    NT = T // 128            # token/key tiles (full seq)
    QB = T // 512            # q blocks (full seq)
    TOWN = T // 2            # own tokens
    NTO = TOWN // 128
    G = 4                    # 2-head groups per core
    HO = 8                   # own heads
    TBW = 512                # token-block width for MLP-up matmuls
    NTB = TOWN // TBW
    SCALE = 1.0 / float(np.sqrt(D))
    EXPSCALE = SCALE / (WS * XS) ** 2
    EXPBIAS = float(np.log(ES))

    nc = bacc.Bacc("TRN2", target_bir_lowering=False, debug=False,
                   num_devices=N_CORES)

    # ---- external inputs ----
    x_full = nc.dram_tensor("x_full", [T, D], F32, kind="ExternalInput").ap()
    x_own = nc.dram_tensor("x_own", [TOWN, D], F32, kind="ExternalInput").ap()
    wq_d = nc.dram_tensor("wq", [G, DC, 128, 128], F8, kind="ExternalInput").ap()
    wk_d = nc.dram_tensor("wk", [G, DC, 128, 128], F8, kind="ExternalInput").ap()
    wv_d = nc.dram_tensor("wv", [DC, 128, HO * 64], BF16, kind="ExternalInput").ap()
    wp_d = nc.dram_tensor("wp", [DC, 128, D], BF16, kind="ExternalInput").ap()
    w1_d = nc.dram_tensor("w1", [DC, 128, DFF], BF16, kind="ExternalInput").ap()
    b1_d = nc.dram_tensor("b1", [128, HS], F32, kind="ExternalInput").ap()
    w2_d = nc.dram_tensor("w2", [HS, 128, D], BF16, kind="ExternalInput").ap()
    if mask_mode == "causal":
        maskt_d = nc.dram_tensor("maskt", [4, 128, 512], BF16,
                                 kind="ExternalInput").ap()
    elif mask_mode == "general":
        maskt_d = nc.dram_tensor("maskt", [NT, 128, T], BF16,
                                 kind="ExternalInput").ap()
    cond_d = {}
    for nm in ("g1b", "beta1b", "bpb", "g2b", "beta2b", "b2b"):
        if nm in flags:
            cond_d[nm] = nc.dram_tensor(nm, [128, D], F32,
                                        kind="ExternalInput").ap()

    sel2_d = nc.dram_tensor("sel2", [2, 128], BF16, kind="ExternalInput").ap()
    out_d = nc.dram_tensor("out", [TOWN, D], F32, kind="ExternalOutput").ap()

    # ---- internal DRAM for the two split collectives ----
    stage0_d = nc.dram_tensor("stage0", [260, T], BF16).ap()
    stage1_d = nc.dram_tensor("stage1", [260, T], BF16).ap()
    ag0_d = nc.dram_tensor("agout0", [2 * 260, T], BF16).ap()
    ag1_d = nc.dram_tensor("agout1", [2 * 260, T], BF16).ap()

    replica_groups = [[2 * i, 2 * i + 1] for i in range(N_CORES // 2)]

    def kts_of(qb):
        return NT if mask_mode == "general" else 4 * (qb + 1)

    with tile.TileContext(nc, pool_alloc_mode="queue") as tc:
        with ExitStack() as octx:
            # ---------------- constants ----------------
            cpool = octx.enter_context(tc.tile_pool(name="const", bufs=1))
            ident = cpool.tile([128, 128], BF16, tag="ident")
            make_identity(nc, ident[:])
            sel2 = cpool.tile([2, 128], BF16, tag="sel2")
            nc.sync.dma_start(sel2[:], sel2_d[:])
            eps_sb = cpool.tile([128, 1], F32, tag="eps")
            nc.vector.memset(eps_sb[:], EPS)
            expb_sb = cpool.tile([128, 1], F32, tag="expb")
            nc.vector.memset(expb_sb[:], EXPBIAS)
            b1_sb = cpool.tile([128, HS], F32, tag="b1")
            nc.sync.dma_start(b1_sb[:], b1_d[:])
            cond_sb = {}
            for nm, d in cond_d.items():
                cond_sb[nm] = cpool.tile([128, D], F32, tag=nm)
                nc.sync.dma_start(cond_sb[nm][:], d[:])
            if mask_mode == "causal":
                maskt_sb = cpool.tile([128, 4, 512], BF16, tag="masktc")
                nc.sync.dma_start(maskt_sb[:],
                                  maskt_d.rearrange("r p f -> p r f"))
            elif mask_mode == "general":
                maskt_sb = cpool.tile([128, NT, T], BF16, tag="masktg")
                nc.sync.dma_start(maskt_sb[:],
                                  maskt_d.rearrange("k p f -> p k f"))

            rank = nc.sync.cc_rank(replica_groups)

            # ---------------- long-lived activation buffers ----------------
            s_keep = ExitStack()    # closed at the very end
            pkeep = s_keep.enter_context(tc.tile_pool(name="keep", bufs=1))
            x1own = pkeep.tile([128, NTO, D], BF16, tag="x1own")
            x3bf = pkeep.tile([128, NTO, D], BF16, tag="x3bf")
            x3T = pkeep.tile([128, DC, TOWN], BF16, tag="x3T")

            # ======== Phase 1+2: LN1 + QKV + attention ========
            s_att = ExitStack()
            pA = s_att.enter_context(tc.tile_pool(name="attkeep", bufs=1))
            x1T = pA.tile([128, DC, T], F8, tag="x1T")
            qt = [pA.tile([128, T], BF16, tag=f"qt{g}", name=f"qt{g}")
                  for g in range(G)]
            kt = [pA.tile([128, T], BF16, tag=f"kt{g}", name=f"kt{g}")
                  for g in range(G)]
            v_aug = pA.tile([128, NT, HO, 65], BF16, tag="vaug")
            wqk_sb = []
            for g in range(G):
                wq_sb = pA.tile([128, DC, 128], F8, tag=f"wq{g}")
                nc.sync.dma_start(wq_sb[:], wq_d[g].rearrange("c p j -> p c j"))
                wk_sb = pA.tile([128, DC, 128], F8, tag=f"wk{g}")
                nc.sync.dma_start(wk_sb[:], wk_d[g].rearrange("c p j -> p c j"))
                wqk_sb.append((wq_sb, wk_sb))
            wv_sb = pA.tile([128, DC, HO * 64], BF16, tag="wv")
            nc.sync.dma_start(wv_sb[:], wv_d.rearrange("c p j -> p c j"))
            nc.vector.memset(v_aug[:, :, :, 64:65], 1.0)

            with ExitStack() as s1:
                p1 = s1.enter_context(tc.tile_pool(name="p1", bufs=2))
                px1bf = s1.enter_context(tc.tile_pool(name="x1bf", bufs=1))
                ps_tp = s1.enter_context(
                    tc.tile_pool(name="ps_tp", bufs=2, space="PSUM"))
                ps_qkv = s1.enter_context(
                    tc.tile_pool(name="ps_qkv", bufs=3, space="PSUM"))
                x1bf = px1bf.tile([128, NT, D], BF16, tag="x1bf")

                def ln1_tile(t):
                    xt = p1.tile([128, D], F32, tag="xt")
                    nc.sync.dma_start(xt[:], x_full[t * 128:(t + 1) * 128, :])
                    stats = p1.tile([128, D // 512, 6], F32, tag="st")
                    for k in range(D // 512):
                        nc.vector.bn_stats(stats[:, k, :],
                                           xt[:, k * 512:(k + 1) * 512])
                    mv = p1.tile([128, 2], F32, tag="mv")
                    nc.vector.bn_aggr(mv[:], stats[:])
                    std = p1.tile([128, 1], F32, tag="sd")
                    nc.scalar.activation(std[:], mv[:, 1:2], AF.Sqrt,
                                         bias=eps_sb[:])
                    rstd = p1.tile([128, 1], F32, tag="rs")
                    nc.vector.reciprocal(rstd[:], std[:])
                    if "g1b" not in flags and "beta1b" not in flags:
                        nc.vector.tensor_scalar(
                            out=x1bf[:, t, :], in0=xt[:], scalar1=mv[:, 0:1],
                            scalar2=rstd[:], op0=ALU.subtract, op1=ALU.mult)
                    else:
                        xh = p1.tile([128, D], F32, tag="xh")
                        nc.vector.tensor_scalar(
                            out=xh[:], in0=xt[:], scalar1=mv[:, 0:1],
                            scalar2=rstd[:], op0=ALU.subtract, op1=ALU.mult)
                        if "g1b" in flags:
                            nc.vector.tensor_mul(xh[:], xh[:], cond_sb["g1b"][:])
                        if "beta1b" in flags:
                            nc.vector.tensor_add(xh[:], xh[:],
                                                 cond_sb["beta1b"][:])
                        nc.vector.tensor_copy(x1bf[:, t, :], xh[:])
                    # transpose 8 chunks (bf16); evac to fp8 x1T (for Q/K
                    # DoubleRow) and to a per-tile bf16 scratch (for V)
                    x1s = p1.tile([128, DC, 128], BF16, tag="x1s")
                    for cb in range(2):
                        tp = ps_tp.tile([128, 512], BF16, tag="tp")
                        for j in range(4):
                            c = cb * 4 + j
                            nc.tensor.transpose(
                                tp[:, j * 128:(j + 1) * 128],
                                x1bf[:, t, c * 128:(c + 1) * 128], ident[:])
                        nc.vector.tensor_scalar(
                            out=x1T[:, cb * 4:(cb + 1) * 4,
                                    t * 128:(t + 1) * 128],
                            in0=tp[:].rearrange("p (c w) -> p c w", c=4),
                            scalar1=XS, scalar2=None, op0=ALU.mult)
                        nc.vector.tensor_copy(
                            x1s[:, cb * 4:(cb + 1) * 4, :],
                            tp[:].rearrange("p (c w) -> p c w", c=4))
                    return x1s

                for qb in range(QB):
                    for t in range(4 * qb, 4 * qb + 4):
                        x1s = ln1_tile(t)
                        # V for this token tile (bf16 for accuracy)
                        vp = ps_qkv.tile([128, 512], F32, tag="vp")
                        for c in range(DC):
                            nc.tensor.matmul(
                                vp[:], x1s[:, c, :], wv_sb[:, c, :],
                                start=(c == 0), stop=(c == DC - 1))
                        nc.vector.tensor_copy(
                            v_aug[:, t, :, 0:64],
                            vp[:].rearrange("p (h j) -> p h j", h=HO))

                # own-half residual: LN recomputed from x_own input
                for to in range(NTO):
                    xt = p1.tile([128, D], F32, tag="xt")
                    nc.sync.dma_start(xt[:],
                                      x_own[to * 128:(to + 1) * 128, :])
                    stats = p1.tile([128, D // 512, 6], F32, tag="st")
                    for k in range(D // 512):
                        nc.vector.bn_stats(stats[:, k, :],
                                           xt[:, k * 512:(k + 1) * 512])
                    mv = p1.tile([128, 2], F32, tag="mv")
                    nc.vector.bn_aggr(mv[:], stats[:])
                    std = p1.tile([128, 1], F32, tag="sd")
                    nc.scalar.activation(std[:], mv[:, 1:2], AF.Sqrt,
                                         bias=eps_sb[:])
                    rstd = p1.tile([128, 1], F32, tag="rs")
                    nc.vector.reciprocal(rstd[:], std[:])
                    if "g1b" not in flags and "beta1b" not in flags:
                        nc.vector.tensor_scalar(
                            out=x1own[:, to, :], in0=xt[:],
                            scalar1=mv[:, 0:1], scalar2=rstd[:],
                            op0=ALU.subtract, op1=ALU.mult)
                    else:
                        xh = p1.tile([128, D], F32, tag="xh")
                        nc.vector.tensor_scalar(
                            out=xh[:], in0=xt[:], scalar1=mv[:, 0:1],
                            scalar2=rstd[:], op0=ALU.subtract, op1=ALU.mult)
                        if "g1b" in flags:
                            nc.vector.tensor_mul(xh[:], xh[:],
                                                 cond_sb["g1b"][:])
                        if "beta1b" in flags:
                            nc.vector.tensor_add(xh[:], xh[:],
                                                 cond_sb["beta1b"][:])
                        nc.vector.tensor_copy(x1own[:, to, :], xh[:])

            # ---- attention per group, collective issued per group ----
            with ExitStack() as s2:
                pE = s2.enter_context(tc.tile_pool(name="ep", bufs=4))
                pAN = s2.enter_context(tc.tile_pool(name="an", bufs=4))
                ps_s = s2.enter_context(
                    tc.tile_pool(name="ps_s", bufs=2, space="PSUM"))
                ps_qk2 = s2.enter_context(
                    tc.tile_pool(name="ps_qk2", bufs=2, space="PSUM"))
                ps_av = s2.enter_context(
                    tc.tile_pool(name="ps_av", bufs=2, space="PSUM"))

                for g in range(G):
                    # Q/K for this group (fp8 DR): dense PE burst that also
                    # keeps the tensor-engine DVFS ramp hot between groups
                    for qb in range(QB):
                        for w_sb, dst in ((wqk_sb[g][0], qt[g]),
                                          (wqk_sb[g][1], kt[g])):
                            pp = ps_qk2.tile([128, 512], F32, tag="qk2")
                            for p in range(DP):
                                nc.tensor.matmul(
                                    pp[:], w_sb[:, 2 * p:2 * p + 2, :],
                                    x1T[:, 2 * p:2 * p + 2,
                                        qb * 512:(qb + 1) * 512],
                                    start=(p == 0), stop=(p == DP - 1),
                                    perf_mode=DR)
                            nc.vector.tensor_copy(
                                dst[:, qb * 512:(qb + 1) * 512], pp[:])
                    for qb in range(QB):
                        nkt = kts_of(qb)
                        av_ps = [ps_av.tile([65, 512], F32, tag="av",
                                            name=f"av{_h}")
                                 for _h in range(2)]
                        for j in range(nkt // 2):
                            s_ps = [ps_s.tile([128, 1024], F32, tag="s",
                                              name=f"s{_h}")
                                    for _h in range(2)]
                            epair = [pE.tile([128, 1024], BF16, tag="ep",
                                             name=f"ep{_h}")
                                     for _h in range(2)]
                            for jj in range(2):
                                ktile = 2 * j + jj
                                for h in range(2):
                                    nc.tensor.matmul(
                                        s_ps[h][:, jj * 512:(jj + 1) * 512],
                                        kt[g][h * 64:(h + 1) * 64,
                                              ktile * 128:(ktile + 1) * 128],
                                        qt[g][h * 64:(h + 1) * 64,
                                              qb * 512:(qb + 1) * 512],
                                        start=True, stop=True,
                                        tile_position=(h * 64, 0))
                            # additive -inf mask on the PSUM scores, pre-exp
                            for jj in range(2):
                                ktile = 2 * j + jj
                                esl = (slice(None),
                                       slice(jj * 512, (jj + 1) * 512))
                                for h in range(2):
                                    if mask_mode == "causal":
                                        r = ktile - 4 * qb
                                        if r >= 0:
                                            nc.vector.tensor_add(
                                                s_ps[h][esl], s_ps[h][esl],
                                                maskt_sb[:, r, :])
                                    elif mask_mode == "general":
                                        nc.vector.tensor_add(
                                            s_ps[h][esl], s_ps[h][esl],
                                            maskt_sb[:, ktile,
                                                     qb * 512:(qb + 1) * 512])
                            for h in range(2):
                                nc.scalar.activation(epair[h][:], s_ps[h][:],
                                                     AF.Exp, scale=EXPSCALE,
                                                     bias=expb_sb[:])
                            for jj in range(2):
                                ktile = 2 * j + jj
                                for h in range(2):
                                    hl = 2 * g + h
                                    nc.tensor.matmul(
                                        av_ps[h][:],
                                        v_aug[:, ktile, hl, :],
                                        epair[h][:, jj * 512:(jj + 1) * 512],
                                        start=(ktile == 0),
                                        stop=(ktile == nkt - 1))
                        for h in range(2):
                            part, gl = g // 2, g % 2
                            st = stage0_d if part == 0 else stage1_d
                            an = pAN.tile([65, 512], BF16, tag="an")
                            nc.vector.tensor_copy(an[:], av_ps[h][:])
                            row = (gl * 2 + h) * 64
                            nc.sync.dma_start(
                                st[row:row + 64, qb * 512:(qb + 1) * 512],
                                an[0:64, :])
                            nc.sync.dma_start(
                                st[256 + gl * 2 + h:256 + gl * 2 + h + 1,
                                   qb * 512:(qb + 1) * 512],
                                an[64:65, :])
                    if g == 1:
                        nc.gpsimd.collective_compute(
                            "AllGather", ALU.bypass,
                            replica_groups=replica_groups,
                            ins=[stage0_d[:]], outs=[ag0_d[:]])
                    elif g == 3:
                        nc.gpsimd.collective_compute(
                            "AllGather", ALU.bypass,
                            replica_groups=replica_groups,
                            ins=[stage1_d[:]], outs=[ag1_d[:]])

            s_att.close()

            # ======== Phase 3: proj + residual + LN2 ========
            with ExitStack() as s3:
                p4 = s3.enter_context(tc.tile_pool(name="p4", bufs=3))
                pAG = s3.enter_context(tc.tile_pool(name="ag", bufs=1))
                pX2 = s3.enter_context(tc.tile_pool(name="x2", bufs=1))
                ps_p4 = s3.enter_context(
                    tc.tile_pool(name="ps_p4", bufs=4, space="PSUM"))
                ps_zb = s3.enter_context(
                    tc.tile_pool(name="ps_zb", bufs=1, space="PSUM"))
                ps_t2 = s3.enter_context(
                    tc.tile_pool(name="ps_t2", bufs=2, space="PSUM"))

                ag = pAG.tile([128, DC, TOWN], BF16, tag="ag")
                agv0 = ag0_d.rearrange("d (h t) -> d h t", h=2)
                agv1 = ag1_d.rearrange("d (h t) -> d h t", h=2)
                for c in range(DC):
                    H0 = 2 * c
                    r, hh = H0 // 8, H0 % 8
                    agv = agv0 if hh < 4 else agv1
                    row = r * 260 + (hh % 4) * 64
                    nc.sync.dma_start(
                        ag[:, c, :],
                        agv[row:row + 128, bass.ds(rank, 1), :])
                # Z rows: even heads -> partitions 0-7, odd -> 8-15
                zr = pAG.tile([16, TOWN], BF16, tag="zr")
                for r in range(2):
                    for p_, agv in ((0, agv0), (1, agv1)):
                        base = r * 260 + 256
                        rows = agv[base:base + 4, bass.ds(rank, 1), :]
                        rows = rows.rearrange("(h two) o t -> two h o t",
                                              two=2)
                        dst0 = r * 4 + p_ * 2
                        nc.sync.dma_start(
                            zr[dst0:dst0 + 2, :], rows[0:1, :, :, :])
                        nc.sync.dma_start(
                            zr[8 + dst0:8 + dst0 + 2, :], rows[1:2, :, :, :])
                zrf = pAG.tile([16, TOWN], F32, tag="zrf")
                with nc.allow_low_precision(reason="z recip"):
                    nc.vector.reciprocal(zrf[:], zr[:])
                zrb = pAG.tile([16, TOWN], BF16, tag="zrb")
                nc.vector.tensor_copy(zrb[:], zrf[:])
                zflat = pAG.tile([2, 8 * TOWN], BF16, tag="zflat")
                nc.sync.dma_start(
                    zflat[0:1, :].rearrange("o (h t) -> o h t", h=8),
                    zrb[0:8, :])
                nc.sync.dma_start(
                    zflat[1:2, :].rearrange("o (h t) -> o h t", h=8),
                    zrb[8:16, :])
                wp_sb = pAG.tile([128, DC, D], BF16, tag="wp")
                nc.sync.dma_start(wp_sb[:], wp_d.rearrange("c p e -> p c e"))

                # agn = ag / Z  (bf16 -- proj stays bf16 for accuracy)
                agn = pAG.tile([128, DC, TOWN], BF16, tag="agn")
                for c in range(DC):
                    zbp = ps_zb.tile([128, TOWN], F32, tag="zb")
                    for nb in range(0, TOWN, 512):
                        nc.tensor.matmul(
                            zbp[:, nb:nb + 512], sel2[:],
                            zflat[:, c * TOWN + nb:c * TOWN + nb + 512],
                            start=True, stop=True)
                    zbs = p4.tile([128, TOWN], BF16, tag="zbs")
                    nc.vector.tensor_copy(zbs[:], zbp[:])
                    nc.vector.tensor_mul(agn[:, c, :], ag[:, c, :], zbs[:])

                x2 = pX2.tile([128, NTO, D], F32, tag="x2")
                for t in range(NTO):
                    for eb in range(D // 512):
                        pp = ps_p4.tile([128, 512], F32, tag="pj")
                        for c in range(DC):
                            nc.tensor.matmul(
                                pp[:],
                                agn[:, c, t * 128:(t + 1) * 128],
                                wp_sb[:, c, eb * 512:(eb + 1) * 512],
                                start=(c == 0), stop=(c == DC - 1))
                        if "bpb" in flags:
                            nc.vector.tensor_add(
                                pp[:], pp[:],
                                cond_sb["bpb"][:, eb * 512:(eb + 1) * 512])
                        nc.vector.tensor_add(
                            x2[:, t, eb * 512:(eb + 1) * 512], pp[:],
                            x1own[:, t, eb * 512:(eb + 1) * 512])
                    stats = p4.tile([128, D // 512, 6], F32, tag="st2")
                    for k in range(D // 512):
                        nc.vector.bn_stats(stats[:, k, :],
                                           x2[:, t, k * 512:(k + 1) * 512])
                    mv = p4.tile([128, 2], F32, tag="mv2")
                    nc.vector.bn_aggr(mv[:], stats[:])
                    std = p4.tile([128, 1], F32, tag="sd2")
                    nc.scalar.activation(std[:], mv[:, 1:2], AF.Sqrt,
                                         bias=eps_sb[:])
                    rstd = p4.tile([128, 1], F32, tag="rs2")
                    nc.vector.reciprocal(rstd[:], std[:])
                    if "g2b" not in flags and "beta2b" not in flags:
                        nc.vector.tensor_scalar(
                            out=x3bf[:, t, :], in0=x2[:, t, :],
                            scalar1=mv[:, 0:1], scalar2=rstd[:],
                            op0=ALU.subtract, op1=ALU.mult)
                    else:
                        xh = p4.tile([128, D], F32, tag="xh2")
                        nc.vector.tensor_scalar(
                            out=xh[:], in0=x2[:, t, :], scalar1=mv[:, 0:1],
                            scalar2=rstd[:], op0=ALU.subtract, op1=ALU.mult)
                        if "g2b" in flags:
                            nc.vector.tensor_mul(xh[:], xh[:],
                                                 cond_sb["g2b"][:])
                        if "beta2b" in flags:
                            nc.vector.tensor_add(xh[:], xh[:],
                                                 cond_sb["beta2b"][:])
                        nc.vector.tensor_copy(x3bf[:, t, :], xh[:])
                    for cb in range(2):
                        tp = ps_t2.tile([128, 512], BF16, tag="tp2")
                        for j in range(4):
                            c = cb * 4 + j
                            nc.tensor.transpose(
                                tp[:, j * 128:(j + 1) * 128],
                                x3bf[:, t, c * 128:(c + 1) * 128], ident[:])
                        nc.vector.tensor_copy(
                            x3T[:, cb * 4:(cb + 1) * 4,
                                t * 128:(t + 1) * 128],
                            tp[:].rearrange("p (c w) -> p c w", c=4))

            # ======== Phase 4: MLP up ========
            s_hT = ExitStack()
            phT = s_hT.enter_context(tc.tile_pool(name="hTp", bufs=1))
            hT = phT.tile([128, HS, TOWN], BF16, tag="hT")

            with ExitStack() as s4:
                pW1 = s4.enter_context(tc.tile_pool(name="w1", bufs=3))
                ps_h = s4.enter_context(
                    tc.tile_pool(name="ps_h", bufs=3, space="PSUM"))
                for hs in range(HS):
                    w1_sb = pW1.tile([128, DC, 128], BF16, tag="w1")
                    nc.sync.dma_start(
                        w1_sb[:],
                        w1_d[:, :, hs * 128:(hs + 1) * 128]
                        .rearrange("c p f -> p c f"))
                    for tb in range(NTB):
                        hp = ps_h.tile([128, TBW], F32, tag="h")
                        for c in range(DC):
                            nc.tensor.matmul(
                                hp[:], w1_sb[:, c, :],
                                x3T[:, c, tb * TBW:(tb + 1) * TBW],
                                start=(c == 0), stop=(c == DC - 1))
                        nc.scalar.activation(
                            hT[:, hs, tb * TBW:(tb + 1) * TBW], hp[:],
                            AF.Relu, bias=b1_sb[:, hs:hs + 1])

            # ======== Phase 5: MLP down + output ========
            with ExitStack() as s5:
                pW2 = s5.enter_context(tc.tile_pool(name="w2", bufs=3))
                pO = s5.enter_context(tc.tile_pool(name="o", bufs=4))
                ps_o = s5.enter_context(
                    tc.tile_pool(name="ps_o", bufs=NTO, space="PSUM"))

                for eb in range(D // 512):
                    ops = [ps_o.tile([128, 512], F32, tag="o", name=f"o{_t}")
                           for _t in range(NTO)]
                    for hs in range(HS):
                        w2_sb = pW2.tile([128, 512], BF16, tag="w2")
                        nc.sync.dma_start(
                            w2_sb[:],
                            w2_d[hs, :, eb * 512:(eb + 1) * 512])
                        for t in range(NTO):
                            nc.tensor.matmul(
                                ops[t][:],
                                hT[:, hs, t * 128:(t + 1) * 128],
                                w2_sb[:], start=(hs == 0),
                                stop=(hs == HS - 1))
                    for t in range(NTO):
                        osb = pO.tile([128, 512], F32, tag="osb")
                        nc.vector.tensor_add(
                            osb[:], ops[t][:],
                            x3bf[:, t, eb * 512:(eb + 1) * 512])
                        if "b2b" in flags:
                            nc.vector.tensor_add(
                                osb[:], osb[:],
                                cond_sb["b2b"][:, eb * 512:(eb + 1) * 512])
                        nc.sync.dma_start(
                            out_d[t * 128:(t + 1) * 128,
                                  eb * 512:(eb + 1) * 512], osb[:])
            s_hT.close()
            s_keep.close()

    nc.compile()
    return nc


# ---------------------------------------------------------------------------
# host side
# ---------------------------------------------------------------------------

_PROG_CACHE = {}


def _get_program(T, D, DFF, mask_mode, flags):
    key = (T, D, DFF, mask_mode, tuple(sorted(flags)))
    if key not in _PROG_CACHE:
        _PROG_CACHE[key] = build_program(T, D, DFF, mask_mode, flags)
    return _PROG_CACHE[key]


def _bf(a):
    return np.ascontiguousarray(a.astype(ml_dtypes.bfloat16))


def _f8(a, scale):
    return np.ascontiguousarray(
        np.clip(np.asarray(a, np.float32) * scale, -240.0, 240.0)
        .astype(ml_dtypes.float8_e4m3))


def detect_mask_mode(eos_mask, T):
    em = np.asarray(eos_mask)
    tril = np.tril(np.ones((T, T), em.dtype))
    if all(np.array_equal(em[b], tril) for b in range(em.shape[0])):
        return "causal"
    return "general"


def prepare_inputs(inputs, T, D, DFF, mask_mode):
    """Build the 8 per-core input maps + the enabled general-path flags."""
    DH = 64
    x = np.asarray(inputs["x"], np.float32)
    Wq = np.asarray(inputs["Wq"], np.float32)
    Wk = np.asarray(inputs["Wk"], np.float32)
    Wv = np.asarray(inputs["Wv"], np.float32)
    Wp = np.asarray(inputs["Wp"], np.float32)
    W1 = np.asarray(inputs["W1"], np.float32)
    W2 = np.asarray(inputs["W2"], np.float32)
    DC, HS = D // 128, DFF // 128

    flags = set()
    for nm, bad in (
        ("g1b", not np.all(np.asarray(inputs["g1"]) == 1)),
        ("beta1b", bool(np.any(np.asarray(inputs["beta1"])))),
        ("bpb", bool(np.any(np.asarray(inputs["bp"])))),
        ("g2b", not np.all(np.asarray(inputs["g2"]) == 1)),
        ("beta2b", bool(np.any(np.asarray(inputs["beta2"])))),
        ("b2b", bool(np.any(np.asarray(inputs["b2"])))),
    ):
        if bad:
            flags.add(nm)

    wp_h = _bf(Wp.reshape(DC, 128, D))
    w1_h = _bf(W1.reshape(DC, 128, DFF))
    b1_h = np.ascontiguousarray(
        np.asarray(inputs["b1"], np.float32).reshape(HS, 128).T)
    w2_h = _bf(W2.reshape(HS, 128, D))

    cond_vals = {}
    for nm, src in (("g1b", "g1"), ("beta1b", "beta1"), ("bpb", "bp"),
                    ("g2b", "g2"), ("beta2b", "beta2"), ("b2b", "b2")):
        if nm in flags:
            cond_vals[nm] = np.ascontiguousarray(np.broadcast_to(
                np.asarray(inputs[src], np.float32)[None, :], (128, D)))

    if mask_mode == "causal":
        r_ = np.arange(4)[:, None, None]
        p_ = np.arange(128)[None, :, None]
        f_ = np.arange(512)[None, None, :]
        maskt = _bf(np.where(128 * r_ + p_ <= f_, 0.0, -1e30)
                    .astype(np.float32))

    sel2 = np.zeros((2, 128), np.float32)
    sel2[0, 0:64] = 1.0
    sel2[1, 64:128] = 1.0
    sel2 = _bf(sel2)

    in_maps = []
    for core in range(N_CORES):
        b, half = core // 2, core % 2
        heads = range(8 * half, 8 * half + 8)
        wq_c = np.empty((4, DC, 128, 128), np.float32)
        wk_c = np.empty((4, DC, 128, 128), np.float32)
        for g in range(4):
            for j, hh in enumerate((8 * half + 2 * g, 8 * half + 2 * g + 1)):
                wq_c[g, :, :, j * 64:(j + 1) * 64] = Wq[hh].reshape(DC, 128, DH)
                wk_c[g, :, :, j * 64:(j + 1) * 64] = Wk[hh].reshape(DC, 128, DH)
        wv_c = np.concatenate([Wv[hh].reshape(DC, 128, DH) for hh in heads],
                              axis=2)
        m = {
            "sel2": sel2,
            "x_full": np.ascontiguousarray(x[b]),
            "x_own": np.ascontiguousarray(
                x[b, half * (T // 2):(half + 1) * (T // 2)]),
            "wq": _f8(wq_c, WS), "wk": _f8(wk_c, WS), "wv": _bf(wv_c),
            "wp": wp_h, "w1": w1_h, "b1": b1_h, "w2": w2_h,
        }
        if mask_mode == "causal":
            m["maskt"] = maskt
        else:
            mb_ = np.asarray(inputs["eos_mask"][b], np.float32)  # [Tq, Tk]
            m["maskt"] = _bf(np.where(
                mb_.T.reshape(T // 128, 128, T) != 0, 0.0, -1e30)
                .astype(np.float32))
        for nm, v in cond_vals.items():
            m[nm] = v
        in_maps.append(m)
    return in_maps, flags


def kernel(**inputs):
    B, T, D = inputs["x"].shape
    DFF = inputs["W1"].shape[1]
    mask_mode = detect_mask_mode(inputs["eos_mask"], T)
    in_maps, flags = prepare_inputs(inputs, T, D, DFF, mask_mode)
    nc = _get_program(T, D, DFF, mask_mode, flags)

    from concourse.bass_utils import run_bass_kernel_spmd
    res = run_bass_kernel_spmd(nc, in_maps, list(range(N_CORES)))

    out = np.empty((B, T, D), np.float32)
    for core in range(N_CORES):
        b, half = core // 2, core % 2
        out[b, half * (T // 2):(half + 1) * (T // 2)] = \
            res.results[core]["out"]
    return out


# revision 47
# speedup vs baseline: 1.2035x; 1.2035x over previous
"""Trainium2 Bass kernel for a dense transformer block (B=4, T=2048, D=1024, H=16).

Sharding: 8 cores = 4 batches x 2-way head split for attention, then 2-way
token split (within each batch pair) for the attention projection + MLP.
The only cross-core communication is one pairwise AllToAll of the normalized
per-head attention outputs (bf16, ~2MB/core).

Per-core dataflow (b = core//2, heads = 8*(core%2) .. +8):
  LN1(x_b) -> x1 (bf16 [T,D] full + f32 own-half)      [DVE]
  PE-transpose x1 -> x1T [D,T] bf16                    [PE + DVE evac]
  QKV: QT/KT per 2-head group [128, T], V [T, 8*65] (65th col = ones)
  attention (S^T layout): S^T[keys,q] = KT.T @ QT  (2 heads row-packed)
     exp on ACT (1/sqrt(D) folded into activation scale), causal masking
     via 0/1 mask tiles on the diagonal band
     attnT[65, q] = [V|1].T @ expS^T  (row 64 = softmax denominator Z)
     normalize: recip(Z) -> PE broadcast -> TT mul -> bf16 -> staging DRAM
  AllToAll (pairs): each core ends with all 16 heads' attnT for its token
     half, d-contiguous.
  proj + residual + LN2 -> x3 bf16, PE-transpose -> x3T
  MLP: hT = relu(W1.T @ x3T + b1), out = hT.T @ W2 + x3, DMA out [T/2, D] f32.
"""

import os
import sys

for _p in ("/opt/trn_rl_repo", "/root/.axon_site/_ro/trn_rl_repo"):
    if os.path.isdir(_p) and _p not in sys.path:
        sys.path.insert(0, _p)

from contextlib import ExitStack

import ml_dtypes
import numpy as np

import concourse.bass as bass
import concourse.mybir as mybir
import concourse.tile as tile
from concourse import bacc
from concourse.masks import make_identity

F32 = mybir.dt.float32
F32R = mybir.dt.float32r
BF16 = mybir.dt.bfloat16
AF = mybir.ActivationFunctionType
ALU = mybir.AluOpType

N_CORES = 8
EPS = 1e-5


def build_program(T=2048, D=1024, DFF=4096, mask_mode="causal", flags=()):
    """Emit the SPMD program (identical across cores; all per-core variation
    comes in via input data). Returns the compiled Bacc object."""
    flags = set(flags)
    DC = D // 128            # d-model 128-chunks
    HS = DFF // 128          # mlp hidden 128-slices
    NT = T // 128            # token/key tiles (full seq)
    QB = T // 512            # q blocks (full seq)
    TOWN = T // 2            # own tokens
    NTO = TOWN // 128
    G = 4                    # 2-head groups per core
    HO = 8                   # own heads
    TBW = min(512, TOWN)     # token-block width for MLP matmuls
    NTB = TOWN // TBW
    SCALE = 1.0 / float(np.sqrt(D))

    nc = bacc.Bacc("TRN2", target_bir_lowering=False, debug=False,
                   num_devices=N_CORES)

    # ---- external inputs ----
    x_full = nc.dram_tensor("x_full", [T, D], F32, kind="ExternalInput").ap()
    x_own = nc.dram_tensor("x_own", [TOWN, D], F32, kind="ExternalInput").ap()
    wq_d = nc.dram_tensor("wq", [G, DC, 128, 128], BF16, kind="ExternalInput").ap()
    wk_d = nc.dram_tensor("wk", [G, DC, 128, 128], BF16, kind="ExternalInput").ap()
    wv_d = nc.dram_tensor("wv", [DC, 128, HO * 64], BF16, kind="ExternalInput").ap()
    wp_d = nc.dram_tensor("wp", [DC, 128, D], BF16, kind="ExternalInput").ap()
    w1_d = nc.dram_tensor("w1", [DC, 128, DFF], BF16, kind="ExternalInput").ap()
    b1_d = nc.dram_tensor("b1", [128, HS], F32, kind="ExternalInput").ap()
    w2_d = nc.dram_tensor("w2", [HS, 128, D], BF16, kind="ExternalInput").ap()
    if mask_mode == "causal":
        maskt_d = nc.dram_tensor("maskt", [4, 128, 512], BF16,
                                 kind="ExternalInput").ap()
    elif mask_mode == "general":
        maskt_d = nc.dram_tensor("maskt", [NT, 128, T], BF16,
                                 kind="ExternalInput").ap()
    cond_d = {}
    for nm in ("g1b", "beta1b", "bpb", "g2b", "beta2b", "b2b"):
        if nm in flags:
            cond_d[nm] = nc.dram_tensor(nm, [128, D], F32,
                                        kind="ExternalInput").ap()

    sel2_d = nc.dram_tensor("sel2", [2, 128], BF16, kind="ExternalInput").ap()
    out_d = nc.dram_tensor("out", [TOWN, D], F32, kind="ExternalOutput").ap()

    # ---- internal DRAM for the collective ----
    stage0_d = nc.dram_tensor("stage0", [260, T], BF16).ap()
    stage1_d = nc.dram_tensor("stage1", [260, T], BF16).ap()
    ag0_d = nc.dram_tensor("agout0", [2 * 260, T], BF16).ap()
    ag1_d = nc.dram_tensor("agout1", [2 * 260, T], BF16).ap()

    replica_groups = [[2 * i, 2 * i + 1] for i in range(N_CORES // 2)]

    def kts_of(qb):
        return NT if mask_mode == "general" else 4 * (qb + 1)

    with tile.TileContext(nc, pool_alloc_mode="queue") as tc:
        with ExitStack() as octx:
            # ---------------- constants ----------------
            cpool = octx.enter_context(tc.tile_pool(name="const", bufs=1))
            ident = cpool.tile([128, 128], BF16, tag="ident")
            make_identity(nc, ident[:])
            # sel2: K=2 selector for broadcasting (headA, headB) recip-Z
            # rows into partitions 0-63 / 64-127 of one matmul output
            sel2 = cpool.tile([2, 128], BF16, tag="sel2")
            nc.sync.dma_start(sel2[:], sel2_d[:])
            eps_sb = cpool.tile([128, 1], F32, tag="eps")
            nc.vector.memset(eps_sb[:], EPS)
            b1_sb = cpool.tile([128, HS], F32, tag="b1")
            nc.sync.dma_start(b1_sb[:], b1_d[:])
            cond_sb = {}
            for nm, d in cond_d.items():
                cond_sb[nm] = cpool.tile([128, D], F32, tag=nm)
                nc.sync.dma_start(cond_sb[nm][:], d[:])
            if mask_mode == "causal":
                maskt_sb = cpool.tile([128, 4, 512], BF16, tag="masktc")
                nc.sync.dma_start(maskt_sb[:],
                                  maskt_d.rearrange("r p f -> p r f"))
            elif mask_mode == "general":
                maskt_sb = cpool.tile([128, NT, T], BF16, tag="masktg")
                nc.sync.dma_start(maskt_sb[:],
                                  maskt_d.rearrange("k p f -> p k f"))

            # ---------------- long-lived activation buffers ----------------
            # LIFO pool nesting: x3bf > x1own > x1T > per-phase pools
            s_x3 = ExitStack()      # closed at the very end
            px3 = s_x3.enter_context(tc.tile_pool(name="x3keep", bufs=1))
            x3bf = px3.tile([128, NTO, D], BF16, tag="x3bf")

            s_x1own = ExitStack()   # closed after the proj/LN2 phase
            px1own = s_x1own.enter_context(tc.tile_pool(name="x1own", bufs=1))
            x1own = px1own.tile([128, NTO, D], F32, tag="x1own")

            s_x1T = ExitStack()     # closed after attention
            px1T = s_x1T.enter_context(tc.tile_pool(name="x1T", bufs=1))
            x1T = px1T.tile([128, DC, T], BF16, tag="x1T")

            # ================ Phase 1: LN1 + transpose ================
            with ExitStack() as sA:
                p1 = sA.enter_context(tc.tile_pool(name="p1", bufs=3))
                px1bf = sA.enter_context(tc.tile_pool(name="x1bf", bufs=1))
                ps_tp = sA.enter_context(
                    tc.tile_pool(name="ps_tp", bufs=2, space="PSUM"))
                x1bf = px1bf.tile([128, NT, D], BF16, tag="x1bf")

                def ln_tile(dst_ap, src_dram_rows, gflag, bflag):
                    xt = p1.tile([128, D], F32, tag="xt")
                    nc.sync.dma_start(xt[:], src_dram_rows)
                    stats = p1.tile([128, D // 512, 6], F32, tag="st")
                    for k in range(D // 512):
                        nc.vector.bn_stats(stats[:, k, :],
                                           xt[:, k * 512:(k + 1) * 512])
                    mv = p1.tile([128, 2], F32, tag="mv")
                    nc.vector.bn_aggr(mv[:], stats[:])
                    std = p1.tile([128, 1], F32, tag="sd")
                    nc.scalar.activation(std[:], mv[:, 1:2], AF.Sqrt, bias=eps_sb[:])
                    rstd = p1.tile([128, 1], F32, tag="rs")
                    nc.vector.reciprocal(rstd[:], std[:])
                    if gflag not in flags and bflag not in flags:
                        nc.vector.tensor_scalar(
                            out=dst_ap, in0=xt[:], scalar1=mv[:, 0:1],
                            scalar2=rstd[:], op0=ALU.subtract, op1=ALU.mult)
                    else:
                        xh = p1.tile([128, D], F32, tag="xh")
                        nc.vector.tensor_scalar(
                            out=xh[:], in0=xt[:], scalar1=mv[:, 0:1],
                            scalar2=rstd[:], op0=ALU.subtract, op1=ALU.mult)
                        if gflag in flags:
                            nc.vector.tensor_mul(xh[:], xh[:], cond_sb[gflag][:])
                        if bflag in flags:
                            nc.vector.tensor_add(xh[:], xh[:], cond_sb[bflag][:])
                        nc.vector.tensor_copy(dst_ap, xh[:])

                for t in range(NT):
                    ln_tile(x1bf[:, t, :], x_full[t * 128:(t + 1) * 128, :],
                            "g1b", "beta1b")
                for t in range(NTO):
                    ln_tile(x1own[:, t, :], x_own[t * 128:(t + 1) * 128, :],
                            "g1b", "beta1b")
                for t in range(NT):
                    for c in range(DC):
                        tp = ps_tp.tile([128, 128], BF16, tag="tp")
                        nc.tensor.transpose(
                            tp[:], x1bf[:, t, c * 128:(c + 1) * 128], ident[:])
                        nc.vector.tensor_copy(
                            x1T[:, c, t * 128:(t + 1) * 128], tp[:])

            # ================ Phases 2+3: QKV + attention ================
            with ExitStack() as sB:
                pV = sB.enter_context(tc.tile_pool(name="vq", bufs=1))
                pQK = sB.enter_context(tc.tile_pool(name="qk", bufs=2))
                pWqk = sB.enter_context(tc.tile_pool(name="wqk", bufs=2))
                pE = sB.enter_context(tc.tile_pool(name="ep", bufs=4))
                pAN = sB.enter_context(tc.tile_pool(name="an", bufs=4))
                ps_s = sB.enter_context(
                    tc.tile_pool(name="ps_s", bufs=2, space="PSUM"))
                ps_av = sB.enter_context(
                    tc.tile_pool(name="ps_av", bufs=2, space="PSUM"))
                ps_misc = sB.enter_context(
                    tc.tile_pool(name="ps_misc", bufs=2, space="PSUM"))

                v_aug = pV.tile([128, NT, HO, 65], BF16, tag="vaug")
                wv_sb = pV.tile([128, DC, HO * 64], BF16, tag="wv")
                nc.sync.dma_start(wv_sb[:], wv_d.rearrange("c p j -> p c j"))
                for t in range(NT):
                    vp = ps_misc.tile([128, 512], F32, tag="misc")
                    for c in range(DC):
                        nc.tensor.matmul(vp[:],
                                         x1T[:, c, t * 128:(t + 1) * 128],
                                         wv_sb[:, c, :], start=(c == 0),
                                         stop=(c == DC - 1))
                    nc.vector.tensor_copy(
                        v_aug[:, t, :, 0:64],
                        vp[:].rearrange("p (h j) -> p h j", h=HO))
                nc.vector.memset(v_aug[:, :, :, 64:65], 1.0)

                for g in range(G):
                    wq_sb = pWqk.tile([128, DC, 128], BF16, tag="wq")
                    nc.sync.dma_start(wq_sb[:],
                                      wq_d[g].rearrange("c p j -> p c j"))
                    wk_sb = pWqk.tile([128, DC, 128], BF16, tag="wk")
                    nc.sync.dma_start(wk_sb[:],
                                      wk_d[g].rearrange("c p j -> p c j"))
                    qt = pQK.tile([128, T], BF16, tag="qt")
                    kt = pQK.tile([128, T], BF16, tag="kt")
                    for qb in range(QB):
                        for w_sb, dst in ((wq_sb, qt), (wk_sb, kt)):
                            pp = ps_misc.tile([128, 512], F32, tag="misc")
                            for c in range(DC):
                                nc.tensor.matmul(
                                    pp[:], w_sb[:, c, :],
                                    x1T[:, c, qb * 512:(qb + 1) * 512],
                                    start=(c == 0), stop=(c == DC - 1))
                            nc.vector.tensor_copy(
                                dst[:, qb * 512:(qb + 1) * 512], pp[:])

                    for qb in range(QB):
                        nkt = kts_of(qb)
                        av_ps = [ps_av.tile([65, 512], F32, tag="av",
                                            name=f"av{_h}")
                                 for _h in range(2)]
                        for j in range(nkt // 2):
                            s_ps = [ps_s.tile([128, 1024], F32, tag="s",
                                              name=f"s{_h}")
                                    for _h in range(2)]
                            epair = [pE.tile([128, 1024], BF16, tag="ep",
                                              name=f"ep{_h}")
                                     for _h in range(2)]
                            for jj in range(2):
                                ktile = 2 * j + jj
                                for h in range(2):
                                    nc.tensor.matmul(
                                        s_ps[h][:, jj * 512:(jj + 1) * 512],
                                        kt[h * 64:(h + 1) * 64,
                                           ktile * 128:(ktile + 1) * 128],
                                        qt[h * 64:(h + 1) * 64,
                                           qb * 512:(qb + 1) * 512],
                                        start=True, stop=True,
                                        tile_position=(h * 64, 0))
                            for h in range(2):
                                nc.scalar.activation(epair[h][:], s_ps[h][:],
                                                     AF.Exp, scale=SCALE)
                            for jj in range(2):
                                ktile = 2 * j + jj
                                esl = (slice(None),
                                       slice(jj * 512, (jj + 1) * 512))
                                for h in range(2):
                                    if mask_mode == "causal":
                                        r = ktile - 4 * qb
                                        if r >= 0:
                                            nc.vector.tensor_mul(
                                                epair[h][esl], epair[h][esl],
                                                maskt_sb[:, r, :])
                                    elif mask_mode == "general":
                                        nc.vector.tensor_mul(
                                            epair[h][esl], epair[h][esl],
                                            maskt_sb[:, ktile,
                                                     qb * 512:(qb + 1) * 512])
                                for h in range(2):
                                    hl = 2 * g + h
                                    nc.tensor.matmul(
                                        av_ps[h][:],
                                        v_aug[:, ktile, hl, :],
                                        epair[h][esl],
                                        start=(ktile == 0),
                                        stop=(ktile == nkt - 1))
                        for h in range(2):
                            part, gl = g // 2, g % 2
                            st = stage0_d if part == 0 else stage1_d
                            an = pAN.tile([65, 512], BF16, tag="an")
                            nc.vector.tensor_copy(an[:], av_ps[h][:])
                            row = (gl * 2 + h) * 64
                            nc.sync.dma_start(
                                st[row:row + 64, qb * 512:(qb + 1) * 512],
                                an[0:64, :])
                            nc.sync.dma_start(
                                st[256 + gl * 2 + h:256 + gl * 2 + h + 1,
                                   qb * 512:(qb + 1) * 512],
                                an[64:65, :])

            s_x1T.close()

            # ---- collectives: pairwise AllGather, split for overlap ----
            # part 0 = own head-groups 0-1 (stage rows 0-255),
            # part 1 = own head-groups 2-3 (rows 256-511)
            nc.gpsimd.collective_compute(
                "AllGather", ALU.bypass, replica_groups=replica_groups,
                ins=[stage0_d[:]], outs=[ag0_d[:]])

            # ================ Phase 4: proj + residual + LN2 ================
            with ExitStack() as sC:
                p4 = sC.enter_context(tc.tile_pool(name="p4", bufs=3))
                pAG = sC.enter_context(tc.tile_pool(name="ag", bufs=1))
                pX2 = sC.enter_context(tc.tile_pool(name="x2", bufs=1))
                ps_p4 = sC.enter_context(
                    tc.tile_pool(name="ps_p4", bufs=4, space="PSUM"))
                ps_zb = sC.enter_context(
                    tc.tile_pool(name="ps_zb", bufs=2, space="PSUM"))

                ag = pAG.tile([128, DC, TOWN], BF16, tag="ag")
                zr = pAG.tile([16, TOWN], BF16, tag="zr")
                rank = nc.sync.cc_rank(replica_groups)
                agv0 = ag0_d.rearrange("d (h t) -> d h t", h=2)
                agv1 = ag1_d.rearrange("d (h t) -> d h t", h=2)

                def load_half(agv, use_agv0):
                    for c in range(DC):
                        H0 = 2 * c                # first head in chunk
                        r, hh = H0 // 8, H0 % 8
                        if (hh < 4) != use_agv0:
                            continue
                        row = r * 260 + (hh % 4) * 64
                        nc.sync.dma_start(
                            ag[:, c, :],
                            agv[row:row + 128, bass.ds(rank, 1), :])
                    # Z rows: head H at partition H//2 + 8*(H%2)
                    p_ = 0 if use_agv0 else 1
                    for r in range(2):
                        base = r * 260 + 256
                        rows = agv[base:base + 4, bass.ds(rank, 1), :]
                        rows = rows.rearrange("(h two) o t -> two h o t",
                                              two=2)
                        dst0 = r * 4 + p_ * 2
                        nc.sync.dma_start(
                            zr[dst0:dst0 + 2, :], rows[0:1, :, :, :])
                        nc.sync.dma_start(
                            zr[8 + dst0:8 + dst0 + 2, :], rows[1:2, :, :, :])

                # stage0's chunks + wp stream while AG1 is still in flight
                load_half(agv0, True)
                wp_early = pAG.tile([128, DC, D], BF16, tag="wp")
                nc.sync.dma_start(wp_early[:], wp_d.rearrange("c p e -> p c e"))
                nc.gpsimd.collective_compute(
                    "AllGather", ALU.bypass, replica_groups=replica_groups,
                    ins=[stage1_d[:]], outs=[ag1_d[:]])
                load_half(agv1, False)
                zrf = pAG.tile([16, TOWN], F32, tag="zrf")
                with nc.allow_low_precision(reason="z recip"):
                    nc.vector.reciprocal(zrf[:], zr[:])
                zrb = pAG.tile([16, TOWN], BF16, tag="zrb")
                nc.vector.tensor_copy(zrb[:], zrf[:])
                # zflat: partition 0 = even heads, partition 1 = odd heads,
                # head-pair c at column block c
                zflat = pAG.tile([2, 8 * TOWN], BF16, tag="zflat")
                nc.sync.dma_start(
                    zflat[0:1, :].rearrange("o (h t) -> o h t", h=8),
                    zrb[0:8, :])
                nc.sync.dma_start(
                    zflat[1:2, :].rearrange("o (h t) -> o h t", h=8),
                    zrb[8:16, :])
                wp_sb = wp_early

                agn = pAG.tile([128, DC, TOWN], BF16, tag="agn")
                for c in range(DC):
                    zbp = ps_zb.tile([128, TOWN], F32, tag="zb")
                    for nb in range(0, TOWN, 512):
                        nw = min(512, TOWN - nb)
                        nc.tensor.matmul(
                            zbp[:, nb:nb + nw], sel2[:],
                            zflat[:, c * TOWN + nb:c * TOWN + nb + nw],
                            start=True, stop=True)
                    zbs = p4.tile([128, TOWN], BF16, tag="zbs")
                    nc.vector.tensor_copy(zbs[:], zbp[:])
                    nc.vector.tensor_mul(agn[:, c, :], ag[:, c, :], zbs[:])

                x2 = pX2.tile([128, NTO, D], F32, tag="x2")
                for t in range(NTO):
                    for eb in range(D // 512):
                        pp = ps_p4.tile([128, 512], F32, tag="pj")
                        for c in range(DC):
                            nc.tensor.matmul(
                                pp[:], agn[:, c, t * 128:(t + 1) * 128],
                                wp_sb[:, c, eb * 512:(eb + 1) * 512],
                                start=(c == 0), stop=(c == DC - 1))
                        if "bpb" in flags:
                            nc.vector.tensor_add(
                                pp[:], pp[:],
                                cond_sb["bpb"][:, eb * 512:(eb + 1) * 512])
                        nc.vector.tensor_add(
                            x2[:, t, eb * 512:(eb + 1) * 512], pp[:],
                            x1own[:, t, eb * 512:(eb + 1) * 512])
                    stats = p4.tile([128, D // 512, 6], F32, tag="st2")
                    for k in range(D // 512):
                        nc.vector.bn_stats(stats[:, k, :],
                                           x2[:, t, k * 512:(k + 1) * 512])
                    mv = p4.tile([128, 2], F32, tag="mv2")
                    nc.vector.bn_aggr(mv[:], stats[:])
                    std = p4.tile([128, 1], F32, tag="sd2")
                    nc.scalar.activation(std[:], mv[:, 1:2], AF.Sqrt, bias=eps_sb[:])
                    rstd = p4.tile([128, 1], F32, tag="rs2")
                    nc.vector.reciprocal(rstd[:], std[:])
                    if "g2b" not in flags and "beta2b" not in flags:
                        nc.vector.tensor_scalar(
                            out=x3bf[:, t, :], in0=x2[:, t, :],
                            scalar1=mv[:, 0:1], scalar2=rstd[:],
                            op0=ALU.subtract, op1=ALU.mult)
                    else:
                        xh = p4.tile([128, D], F32, tag="xh2")
                        nc.vector.tensor_scalar(
                            out=xh[:], in0=x2[:, t, :], scalar1=mv[:, 0:1],
                            scalar2=rstd[:], op0=ALU.subtract, op1=ALU.mult)
                        if "g2b" in flags:
                            nc.vector.tensor_mul(xh[:], xh[:],
                                                 cond_sb["g2b"][:])
                        if "beta2b" in flags:
                            nc.vector.tensor_add(xh[:], xh[:],
                                                 cond_sb["beta2b"][:])
                        nc.vector.tensor_copy(x3bf[:, t, :], xh[:])
            s_x1own.close()

            s_hT = ExitStack()
            phT = s_hT.enter_context(tc.tile_pool(name="hTp", bufs=1))
            hT = phT.tile([128, HS, TOWN], BF16, tag="hT")

            # ================ Phase 5: MLP up ================
            with ExitStack() as sD:
                pX3T = sD.enter_context(tc.tile_pool(name="x3T", bufs=1))
                pW1 = sD.enter_context(tc.tile_pool(name="w1", bufs=3))
                ps_t2 = sD.enter_context(
                    tc.tile_pool(name="ps_t2", bufs=2, space="PSUM"))
                ps_h = sD.enter_context(
                    tc.tile_pool(name="ps_h", bufs=3, space="PSUM"))

                x3T = pX3T.tile([128, DC, TOWN], BF16, tag="x3T")
                for t in range(NTO):
                    for c in range(DC):
                        tp = ps_t2.tile([128, 128], BF16, tag="tp2")
                        nc.tensor.transpose(
                            tp[:], x3bf[:, t, c * 128:(c + 1) * 128], ident[:])
                        nc.vector.tensor_copy(
                            x3T[:, c, t * 128:(t + 1) * 128], tp[:])

                for hs in range(HS):
                    w1_sb = pW1.tile([128, DC, 128], BF16, tag="w1")
                    nc.sync.dma_start(
                        w1_sb[:],
                        w1_d[:, :, hs * 128:(hs + 1) * 128]
                        .rearrange("c p f -> p c f"))
                    for tb in range(NTB):
                        hp = ps_h.tile([128, TBW], F32, tag="h")
                        for c in range(DC):
                            nc.tensor.matmul(
                                hp[:], w1_sb[:, c, :],
                                x3T[:, c, tb * TBW:(tb + 1) * TBW],
                                start=(c == 0), stop=(c == DC - 1))
                        nc.scalar.activation(
                            hT[:, hs, tb * TBW:(tb + 1) * TBW], hp[:],
                            AF.Relu, bias=b1_sb[:, hs:hs + 1])

            # ================ Phase 6: MLP down + output ================
            with ExitStack() as sE:
                pW2 = sE.enter_context(tc.tile_pool(name="w2", bufs=3))
                pO = sE.enter_context(tc.tile_pool(name="o", bufs=3))
                ps_o = sE.enter_context(
                    tc.tile_pool(name="ps_o", bufs=NTO, space="PSUM"))

                TCH = min(8, NTO)  # token tiles per accumulation chunk
                for eb in range(D // 512):
                    for t0 in range(0, NTO, TCH):
                        ops = [ps_o.tile([128, 512], F32, tag="o",
                                         name=f"o{_t}")
                               for _t in range(TCH)]
                        for hs in range(HS):
                            w2_sb = pW2.tile([128, 512], BF16, tag="w2")
                            nc.sync.dma_start(
                                w2_sb[:], w2_d[hs, :, eb * 512:(eb + 1) * 512])
                            for i in range(TCH):
                                t = t0 + i
                                nc.tensor.matmul(
                                    ops[i][:],
                                    hT[:, hs, t * 128:(t + 1) * 128],
                                    w2_sb[:], start=(hs == 0),
                                    stop=(hs == HS - 1))
                        for i in range(TCH):
                            t = t0 + i
                            osb = pO.tile([128, 512], F32, tag="osb")
                            nc.vector.tensor_add(
                                osb[:], ops[i][:],
                                x3bf[:, t, eb * 512:(eb + 1) * 512])
                            if "b2b" in flags:
                                nc.vector.tensor_add(
                                    osb[:], osb[:],
                                    cond_sb["b2b"][:, eb * 512:(eb + 1) * 512])
                            nc.sync.dma_start(
                                out_d[t * 128:(t + 1) * 128,
                                      eb * 512:(eb + 1) * 512], osb[:])
            s_hT.close()
            s_x3.close()

    nc.compile()
    return nc


# ---------------------------------------------------------------------------
# host side
# ---------------------------------------------------------------------------

_PROG_CACHE = {}


def _get_program(T, D, DFF, mask_mode, flags):
    key = (T, D, DFF, mask_mode, tuple(sorted(flags)))
    if key not in _PROG_CACHE:
        _PROG_CACHE[key] = build_program(T, D, DFF, mask_mode, flags)
    return _PROG_CACHE[key]


def _bf(a):
    return np.ascontiguousarray(a.astype(ml_dtypes.bfloat16))


def detect_mask_mode(eos_mask, T):
    em = np.asarray(eos_mask)
    tril = np.tril(np.ones((T, T), em.dtype))
    if all(np.array_equal(em[b], tril) for b in range(em.shape[0])):
        return "causal"
    return "general"


def prepare_inputs(inputs, T, D, DFF, mask_mode):
    """Build the 8 per-core input maps + the enabled general-path flags."""
    DH = 64
    x = np.asarray(inputs["x"], np.float32)
    Wq = np.asarray(inputs["Wq"], np.float32)
    Wk = np.asarray(inputs["Wk"], np.float32)
    Wv = np.asarray(inputs["Wv"], np.float32)
    Wp = np.asarray(inputs["Wp"], np.float32)
    W1 = np.asarray(inputs["W1"], np.float32)
    W2 = np.asarray(inputs["W2"], np.float32)
    DC, HS = D // 128, DFF // 128

    flags = set()
    for nm, bad in (
        ("g1b", not np.all(np.asarray(inputs["g1"]) == 1)),
        ("beta1b", bool(np.any(np.asarray(inputs["beta1"])))),
        ("bpb", bool(np.any(np.asarray(inputs["bp"])))),
        ("g2b", not np.all(np.asarray(inputs["g2"]) == 1)),
        ("beta2b", bool(np.any(np.asarray(inputs["beta2"])))),
        ("b2b", bool(np.any(np.asarray(inputs["b2"])))),
    ):
        if bad:
            flags.add(nm)

    wp_h = _bf(Wp.reshape(DC, 128, D))
    w1_h = _bf(W1.reshape(DC, 128, DFF))
    b1_h = np.ascontiguousarray(
        np.asarray(inputs["b1"], np.float32).reshape(HS, 128).T)
    w2_h = _bf(W2.reshape(HS, 128, D))

    cond_vals = {}
    for nm, src in (("g1b", "g1"), ("beta1b", "beta1"), ("bpb", "bp"),
                    ("g2b", "g2"), ("beta2b", "beta2"), ("b2b", "b2")):
        if nm in flags:
            cond_vals[nm] = np.ascontiguousarray(np.broadcast_to(
                np.asarray(inputs[src], np.float32)[None, :], (128, D)))

    if mask_mode == "causal":
        r_ = np.arange(4)[:, None, None]
        p_ = np.arange(128)[None, :, None]
        f_ = np.arange(512)[None, None, :]
        maskt = _bf((128 * r_ + p_ <= f_).astype(np.float32))

    in_maps = []
    for core in range(N_CORES):
        b, half = core // 2, core % 2
        heads = range(8 * half, 8 * half + 8)
        wq_c = np.empty((4, DC, 128, 128), np.float32)
        wk_c = np.empty((4, DC, 128, 128), np.float32)
        for g in range(4):
            for j, hh in enumerate((8 * half + 2 * g, 8 * half + 2 * g + 1)):
                wq_c[g, :, :, j * 64:(j + 1) * 64] = Wq[hh].reshape(DC, 128, DH)
                wk_c[g, :, :, j * 64:(j + 1) * 64] = Wk[hh].reshape(DC, 128, DH)
        wv_c = np.concatenate([Wv[hh].reshape(DC, 128, DH) for hh in heads],
                              axis=2)
        sel2 = np.zeros((2, 128), np.float32)
        sel2[0, 0:64] = 1.0
        sel2[1, 64:128] = 1.0
        m = {
            "sel2": _bf(sel2),
            "x_full": np.ascontiguousarray(x[b]),
            "x_own": np.ascontiguousarray(
                x[b, half * (T // 2):(half + 1) * (T // 2)]),
            "wq": _bf(wq_c), "wk": _bf(wk_c), "wv": _bf(wv_c),
            "wp": wp_h, "w1": w1_h, "b1": b1_h, "w2": w2_h,
        }
        if mask_mode == "causal":
            m["maskt"] = maskt
        else:
            mb_ = np.asarray(inputs["eos_mask"][b], np.float32)  # [Tq, Tk]
            m["maskt"] = _bf(np.ascontiguousarray(
                mb_.T.reshape(T // 128, 128, T)))
        for nm, v in cond_vals.items():
            m[nm] = v
        in_maps.append(m)
    return in_maps, flags


def kernel(**inputs):
    B, T, D = inputs["x"].shape
    DFF = inputs["W1"].shape[1]
    mask_mode = detect_mask_mode(inputs["eos_mask"], T)
    in_maps, flags = prepare_inputs(inputs, T, D, DFF, mask_mode)
    nc = _get_program(T, D, DFF, mask_mode, flags)

    from concourse.bass_utils import run_bass_kernel_spmd
    res = run_bass_kernel_spmd(nc, in_maps, list(range(N_CORES)))

    out = np.empty((B, T, D), np.float32)
    for core in range(N_CORES):
        b, half = core // 2, core % 2
        out[b, half * (T // 2):(half + 1) * (T // 2)] = \
            res.results[core]["out"]
    return out

